# revision 15
# baseline (speedup 1.0000x reference)
"""DWHT (buggy in-place Walsh-Hadamard channel transform + channel shuffle) on 8 trn2 cores.

The whole nn.Module is a fixed linear map on the channel axis:
    y[b, :, h, w] = T @ x[b, :, h, w]
with T a (512, 256) matrix of small integers (|T| <= 13, exact in fp16).

Structure exploited (variant "t2", default): every row of T is constant on
input-channel pairs, i.e. T = T2 @ A with A the pair-sum map
(s[c] = x[2c] + x[2c+1]) and T2 = T[:, ::2] a (512, 128) small-integer
matrix.  Per sample the kernel therefore:
  1. loads x as fp16 [128 pairs, 2, 784] (one DMA),
  2. computes s = x[:,0] + x[:,1] on DVE (fp16, 2x mode),
  3. runs 8 single-k matmuls T2 @ s (k=128, fp16, exact) into PSUM,
  4. copies PSUM fp32 -> SBUF fp16 (balanced across DVE/ACT/Pool),
  5. DMAs fp16 y tiles out (balanced across the SP/ACT/Pool queues).
Batch 64 is sharded 8-ways (data parallel, 8 samples/core).

Precision: x is rounded to fp16 on the host (T2 and the pair-sum stay exact
in fp16; PSUM accumulates fp32), output is written fp16 and widened to fp32
on the host: measured rel err ~1e-4 vs the fp32 reference, far inside the
2e-2 gate.  The kernel is memory/copy bound: ~9.6 MB of fp16 HBM traffic
per core spread over the three DMA-capable queues, with the PSUM->SBUF
downcast copies load-balanced against them.
"""

import os
import sys

import numpy as np

for _p in ("/opt/trn_rl_repo", "/root/.axon_site/_ro/trn_rl_repo"):
    if os.path.isdir(_p) and _p not in sys.path:
        sys.path.append(_p)

B, C_IN, C_OUT, HH, WW = 64, 256, 512, 28, 28
S = HH * WW  # 784
N_CORES = 8
BS = B // N_CORES  # 8 samples per core
N_PASSES, GROUPS = 8, 8

VARIANT = os.environ.get("DWHT_VARIANT", "t2")

# spatial split per PSUM bank (each chunk <= 512 fp32 = one bank)
N_CHUNKS = ((0, 392), (392, 392))


def _dwht_T() -> np.ndarray:
    """Build the (512, 256) transform matrix by running the reference
    butterfly (including its partial-update in-place semantics) on identity."""
    x = np.zeros((C_OUT, C_IN), np.float64)
    x[:C_IN] = np.eye(C_IN)
    half = C_OUT // 2
    for _ in range(N_PASSES):
        top = x[::2] + x[1::2]
        x = x.copy()
        x[:half] = top
        bottom = x[::2] - x[1::2]
        x[half:] = bottom
    # channel shuffle with groups=8
    x = x.reshape(GROUPS, C_OUT // GROUPS, C_IN).transpose(1, 0, 2).reshape(C_OUT, C_IN)
    return x


# ---------------------------------------------------------------- t2 variant

# engine assignment tables, tuned against the CoreSim cost model.
# copy engines per (sample, m-tile): v=vector(DVE) s=scalar(ACT) g=gpsimd(Pool)
# out-dma queues per (sample, m-tile): y=sync(SP) s=scalar(ACT) g=gpsimd(Pool)
# in-dma queue per sample.
# Balance rationale: ACT does no copies (avoids the one-time activation-table
# load) and serves as a pure DMA queue next to SP; Pool copies are the
# cheapest (no PSUM access bubble in the model) so they take the majority.
_COPY_ENG = os.environ.get(
    "DWHT_COPY", "vggg" + "vggg" + "vggg" + "vvgg" + "vggg" + "vvgg" + "vggg" + "vgvg"
)
_OUT_Q = os.environ.get(
    "DWHT_OUTQ", "ysys" + "yyys" + "ysys" + "yyys" + "ysys" + "ysys" + "ysys" + "yyyy"
)
_IN_Q = os.environ.get("DWHT_INQ", "-ysysyss")


def _build_t2(reps=1):
    import concourse.mybir as mybir
    from concourse import bacc
    from concourse.tile import TileContext

    f32 = mybir.dt.float32
    fp16 = mybir.dt.float16

    nc = bacc.Bacc(None, target_bir_lowering=False)
    x = nc.dram_tensor("x", (BS, C_IN, S), fp16, kind="ExternalInput")
    tt = nc.dram_tensor("tt", (C_IN // 2, C_OUT), fp16, kind="ExternalInput")
    y = nc.dram_tensor("y", (BS, C_OUT, S), fp16, kind="ExternalOutput")

    def q(ch):
        return {"y": nc.sync, "s": nc.scalar, "g": nc.gpsimd, "v": nc.vector}[ch]

    with TileContext(nc) as tc:
        with (
            tc.tile_pool(name="w", bufs=1) as wp,
            tc.tile_pool(name="io", bufs=3) as io,
            tc.tile_pool(name="ps", bufs=4, space="PSUM") as pp,
        ):
            tt2 = wp.tile([128, C_OUT], fp16, tag="tt2")
            nc.sync.dma_start(out=tt2[:], in_=tt[:, :])

            sample_seq = [s for _ in range(reps) for s in range(BS)]
            n_seq = len(sample_seq)
            xss = {}
            # all input loads upfront: input traffic is the only DMA work
            # that exists before the first PSUM tile is ready, so it must
            # fill the queues' lead-in window
            for si in range(n_seq):
                s = sample_seq[si]
                xs = io.tile([128, 2, S], fp16, tag="xs", bufs=BS)
                src = x[s].rearrange("(p two) f -> p two f", two=2)
                if si == 0:
                    # split the first load across two queues so the fill
                    # critical path is one half-transfer, not a full one
                    nc.scalar.dma_start(out=xs[:, :, 0:392], in_=src[:, :, 0:392])
                    nc.sync.dma_start(out=xs[:, :, 392:S], in_=src[:, :, 392:S])
                else:
                    q(_IN_Q[s]).dma_start(out=xs[:], in_=src)
                xss[si] = xs

            for si, s in enumerate(sample_seq):
                xs = xss.pop(si)
                # pair-sum on DVE (fp16, packed, SBUF -> 2x mode); split for
                # the first sample so matmuls can start on the first half
                ss = io.tile([128, S], fp16, tag="ss", bufs=3)
                if si == 0:
                    for n0, nsz in N_CHUNKS:
                        nsl = slice(n0, n0 + nsz)
                        nc.vector.tensor_add(ss[:, nsl], xs[:, 0, nsl], xs[:, 1, nsl])
                else:
                    nc.vector.tensor_add(ss[:], xs[:, 0], xs[:, 1])

                last = si == n_seq - 1
                for m in range(C_OUT // 128):
                    msl = slice(m * 128, (m + 1) * 128)
                    ps = pp.tile([128, 2, 512], f32, tag="ps")
                    for ci, (n0, nsz) in enumerate(N_CHUNKS):
                        nc.tensor.matmul(
                            ps[:, ci, 0:nsz],
                            tt2[:, msl],
                            ss[:, n0 : n0 + nsz],
                            start=True,
                            stop=True,
                        )
                    ysm = io.tile([128, S], fp16, tag="ysm", bufs=6)
                    ysv = ysm.rearrange("p (c n) -> p c n", c=2)
                    if last:
                        # drain fast: per-chunk copies and stores spread over
                        # engines/queues so the tail is one half-tile long
                        cengs = ("v", "g") if m % 2 == 0 else ("g", "v")
                        qouts = ("y", "s", "g", "y", "s", "g", "y", "s")
                        for ci in range(2):
                            dst = ysv[:, ci, 0:392]
                            src = ps[:, ci, 0:392]
                            if cengs[ci] == "v":
                                nc.vector.tensor_copy(dst, src)
                            else:
                                nc.gpsimd.tensor_copy(dst, src)
                            q(qouts[m * 2 + ci]).dma_start(
                                out=y[s, msl, ci * 392 : (ci + 1) * 392],
                                in_=ysm[:, ci * 392 : (ci + 1) * 392],
                            )
                    else:
                        dst = ysv[:, :, 0:392]
                        src = ps[:, :, 0:392]
                        eng = _COPY_ENG[s * 4 + m]
                        if eng == "v":
                            nc.vector.tensor_copy(dst, src)
                        elif eng == "s":
                            nc.scalar.copy(dst, src)
                        else:
                            nc.gpsimd.tensor_copy(dst, src)
                        q(_OUT_Q[s * 4 + m]).dma_start(out=y[s, msl, :], in_=ysm[:])

    nc.compile()
    return nc


# ------------------------------------------------------- fp16_lo (old) variant


def _build_fp16lo(reps=1):
    import concourse.mybir as mybir
    from concourse import bacc
    from concourse.tile import TileContext

    f32 = mybir.dt.float32
    bf16 = mybir.dt.bfloat16
    fp16 = mybir.dt.float16

    nc = bacc.Bacc(None, target_bir_lowering=False)
    x = nc.dram_tensor("x", (BS, C_IN, S), f32, kind="ExternalInput")
    tt = nc.dram_tensor("tt", (C_IN, C_OUT), bf16, kind="ExternalInput")
    y = nc.dram_tensor("y", (BS, C_OUT, S), f32, kind="ExternalOutput")

    with TileContext(nc) as tc:
        with (
            tc.tile_pool(name="w", bufs=1) as wp,
            tc.tile_pool(name="io", bufs=3) as io,
            tc.tile_pool(name="ps", bufs=8, space="PSUM") as pp,
        ):
            tts = []
            tth = []
            for k in range(2):
                t = wp.tile([128, C_OUT], bf16, tag=f"tt{k}")
                nc.sync.dma_start(out=t[:], in_=tt[k * 128 : (k + 1) * 128, :])
                tts.append(t)
                h = wp.tile([128, C_OUT], fp16, tag=f"tth{k}")
                nc.vector.tensor_copy(h[:], t[:])
                tth.append(h)

            sample_seq = [s for _ in range(reps) for s in range(BS)]
            for si, s in enumerate(sample_seq):
                last_sample = si == len(sample_seq) - 1
                first_sample = si == 0
                xsk = []
                for k in range(2):
                    xs = io.tile([128, S], f32, tag="xs", bufs=8)
                    if first_sample:
                        for n0, nsz in N_CHUNKS:
                            nc.scalar.dma_start(
                                out=xs[:, n0 : n0 + nsz],
                                in_=x[s, k * 128 : (k + 1) * 128, n0 : n0 + nsz],
                            )
                    else:
                        nc.scalar.dma_start(
                            out=xs[:], in_=x[s, k * 128 : (k + 1) * 128, :]
                        )
                    xsk.append(xs)

                xhk = [
                    xs.bitcast(bf16).rearrange("p (f two) -> p f two", two=2)[:, :, 1]
                    for xs in xsk
                ]
                xlk = []
                for k in range(2):
                    xl = io.tile([128, S], fp16, tag="xl", bufs=8)
                    if first_sample:
                        for n0, nsz in N_CHUNKS:
                            nsl = slice(n0, n0 + nsz)
                            nc.vector.tensor_sub(
                                xl[:, nsl], xsk[k][:, nsl], xhk[k][:, nsl]
                            )
                    else:
                        nc.vector.tensor_sub(xl[:], xsk[k][:], xhk[k])
                    xlk.append(xl)
                passes = [
                    (xlk[0][:], tth[0]),
                    (xhk[0], tts[0]),
                    (xlk[1][:], tth[1]),
                    (xhk[1], tts[1]),
                ]

                for m in range(C_OUT // 128):
                    msl = slice(m * 128, (m + 1) * 128)
                    ysm = io.tile([128, S], f32, tag="ysm", bufs=6)
                    for ni, (n0, nsz) in enumerate(N_CHUNKS):
                        nsl = slice(n0, n0 + nsz)
                        ps = pp.tile([128, nsz], f32, tag="ps")
                        for i, (src, w) in enumerate(passes):
                            nc.tensor.matmul(
                                ps[:],
                                w[:, msl],
                                src[:, nsl],
                                start=(i == 0),
                                stop=(i == len(passes) - 1),
                            )
                        dst = ysm[:, nsl]
                        if last_sample:
                            if ni == 0:
                                nc.vector.tensor_copy(dst, ps[:])
                                nc.sync.dma_start(out=y[s, msl, nsl], in_=dst)
                            else:
                                nc.scalar.copy(dst, ps[:])
                                nc.scalar.dma_start(out=y[s, msl, nsl], in_=dst)
                        elif (m * len(N_CHUNKS) + ni) % 2 == 0:
                            nc.vector.tensor_copy(dst, ps[:])
                        else:
                            nc.scalar.copy(dst, ps[:])
                    if not last_sample:
                        nc.sync.dma_start(out=y[s, msl, :], in_=ysm[:])

    nc.compile()
    return nc


_cache = {}


def _get_nc(variant=None, reps=1):
    variant = variant or VARIANT
    key = (variant, reps)
    if key not in _cache:
        if variant == "t2":
            _cache[key] = _build_t2(reps)
        elif variant == "fp16_lo":
            _cache[key] = _build_fp16lo(reps)
        else:
            raise ValueError(variant)
    return _cache[key]


def _in_maps(x_np, variant):
    T = _dwht_T()
    if variant == "t2":
        tt_np = np.ascontiguousarray(T[:, ::2].T).astype(np.float16)  # (128, 512)
        x16 = x_np.astype(np.float16)
        return [
            {"x": x16[i * BS : (i + 1) * BS], "tt": tt_np} for i in range(N_CORES)
        ]
    import ml_dtypes

    ttT = np.ascontiguousarray(T.T)  # (256, 512), lhsT layout
    tt_np = ttT.astype(ml_dtypes.bfloat16)
    return [
        {"x": x_np[i * BS : (i + 1) * BS], "tt": tt_np} for i in range(N_CORES)
    ]


def _run(x_np, variant=None, trace=False, reps=1):
    from concourse.bass_utils import run_bass_kernel_spmd

    variant = variant or VARIANT
    nc = _get_nc(variant, reps)
    res = run_bass_kernel_spmd(
        nc, _in_maps(x_np, variant), list(range(N_CORES)), trace=trace
    )
    y = np.stack([r["y"] for r in res.results]).astype(np.float32)
    y = y.reshape(B, C_OUT, HH, WW)
    return y, res


def kernel(x: np.ndarray) -> np.ndarray:
    x_np = np.ascontiguousarray(np.asarray(x), dtype=np.float32).reshape(B, C_IN, S)
    y, _ = _run(x_np)
    return y


# revision 17
# speedup vs baseline: 1.7467x; 1.7467x over previous
"""DWHT (buggy in-place Walsh-Hadamard channel transform + channel shuffle) on 8 trn2 cores.

The whole nn.Module is a fixed linear map on the channel axis:
    y[b, :, h, w] = T @ x[b, :, h, w]
with T a (512, 256) matrix of small integers (|T| <= 13, exact in fp16).

Structure exploited (variant "t2", default): every row of T is constant on
input-channel pairs, i.e. T = T2 @ A with A the pair-sum map
(s[c] = x[2c] + x[2c+1]) and T2 = T[:, ::2] a (512, 128) small-integer
matrix.  Per sample the kernel therefore:
  1. loads x as fp16 [128 pairs, 2, 784] (one DMA),
  2. computes s = x[:,0] + x[:,1] on DVE (fp16, 2x mode),
  3. runs 8 single-k matmuls T2 @ s (k=128, fp16, exact) into PSUM,
  4. copies PSUM fp32 -> SBUF fp16 (balanced across DVE and ACT; gpsimd
     cannot read PSUM, the BIR verifier rejects it),
  5. DMAs fp16 y tiles out (balanced across the SP/ACT/Pool queues).
Batch 64 is sharded 8-ways (data parallel, 8 samples/core).

Precision: x is rounded to fp16 on the host (T2 and the pair-sum stay exact
in fp16; PSUM accumulates fp32), output is written fp16 and widened to fp32
on the host: measured rel err ~1e-4 vs the fp32 reference, far inside the
2e-2 gate.  The kernel is memory/copy bound: ~9.6 MB of fp16 HBM traffic
per core spread over the three DMA-capable queues, with the PSUM->SBUF
downcast copies load-balanced against them.
"""

import os
import sys

import numpy as np

for _p in ("/opt/trn_rl_repo", "/root/.axon_site/_ro/trn_rl_repo"):
    if os.path.isdir(_p) and _p not in sys.path:
        sys.path.append(_p)

B, C_IN, C_OUT, HH, WW = 64, 256, 512, 28, 28
S = HH * WW  # 784
N_CORES = 8
BS = B // N_CORES  # 8 samples per core
N_PASSES, GROUPS = 8, 8

VARIANT = os.environ.get("DWHT_VARIANT", "t2")

# spatial split per PSUM bank (each chunk <= 512 fp32 = one bank)
N_CHUNKS = ((0, 392), (392, 392))


def _dwht_T() -> np.ndarray:
    """Build the (512, 256) transform matrix by running the reference
    butterfly (including its partial-update in-place semantics) on identity."""
    x = np.zeros((C_OUT, C_IN), np.float64)
    x[:C_IN] = np.eye(C_IN)
    half = C_OUT // 2
    for _ in range(N_PASSES):
        top = x[::2] + x[1::2]
        x = x.copy()
        x[:half] = top
        bottom = x[::2] - x[1::2]
        x[half:] = bottom
    # channel shuffle with groups=8
    x = x.reshape(GROUPS, C_OUT // GROUPS, C_IN).transpose(1, 0, 2).reshape(C_OUT, C_IN)
    return x


# ---------------------------------------------------------------- t2 variant

# engine assignment tables, tuned against the CoreSim cost model.
# copy engines per (sample, m-tile): v=vector(DVE) s=scalar(ACT) g=gpsimd(Pool)
# out-dma queues per (sample, m-tile): y=sync(SP) s=scalar(ACT) g=gpsimd(Pool)
# in-dma queue per sample.
# Balance rationale: PSUM->SBUF copies may only run on DVE and ACT (gpsimd
# has no PSUM access), so the SP and Pool queues carry nearly all DMA
# traffic and ACT's queue share goes to copies instead.
_COPY_ENG = os.environ.get(
    "DWHT_COPY", "vsvs" + "vsvs" + "vsvs" + "vsvs" + "vsvs" + "vsvs" + "ssss" + "vsvs"
)
_OUT_Q = os.environ.get(
    "DWHT_OUTQ", "ygyg" + "ygyg" + "ygyg" + "ygyg" + "ygyg" + "ygyg" + "ygyg" + "ygys"
)
_IN_Q = os.environ.get("DWHT_INQ", "-ygygygg")


def _build_t2(reps=1):
    import concourse.mybir as mybir
    from concourse import bacc
    from concourse.tile import TileContext

    f32 = mybir.dt.float32
    fp16 = mybir.dt.float16

    nc = bacc.Bacc(None, target_bir_lowering=False)
    x = nc.dram_tensor("x", (BS, C_IN, S), fp16, kind="ExternalInput")
    tt = nc.dram_tensor("tt", (C_IN // 2, C_OUT), fp16, kind="ExternalInput")
    y = nc.dram_tensor("y", (BS, C_OUT, S), fp16, kind="ExternalOutput")

    def q(ch):
        return {"y": nc.sync, "s": nc.scalar, "g": nc.gpsimd, "v": nc.vector}[ch]

    with TileContext(nc) as tc:
        with (
            tc.tile_pool(name="w", bufs=1) as wp,
            tc.tile_pool(name="io", bufs=3) as io,
            tc.tile_pool(name="ps", bufs=4, space="PSUM") as pp,
        ):
            tt2 = wp.tile([128, C_OUT], fp16, tag="tt2")
            nc.sync.dma_start(out=tt2[:], in_=tt[:, :])

            sample_seq = [s for _ in range(reps) for s in range(BS)]
            n_seq = len(sample_seq)
            xss = {}
            # all input loads upfront: input traffic is the only DMA work
            # that exists before the first PSUM tile is ready, so it must
            # fill the queues' lead-in window
            for si in range(n_seq):
                s = sample_seq[si]
                xs = io.tile([128, 2, S], fp16, tag="xs", bufs=BS)
                src = x[s].rearrange("(p two) f -> p two f", two=2)
                if si == 0:
                    # split the first load across two queues so the fill
                    # critical path is one half-transfer, not a full one
                    nc.sync.dma_start(out=xs[:, :, 0:392], in_=src[:, :, 0:392])
                    nc.gpsimd.dma_start(out=xs[:, :, 392:S], in_=src[:, :, 392:S])
                else:
                    q(_IN_Q[s]).dma_start(out=xs[:], in_=src)
                xss[si] = xs

            for si, s in enumerate(sample_seq):
                xs = xss.pop(si)
                # pair-sum on DVE (fp16, packed, SBUF -> 2x mode); split for
                # the first sample so matmuls can start on the first half
                ss = io.tile([128, S], fp16, tag="ss", bufs=3)
                if si == 0:
                    for n0, nsz in N_CHUNKS:
                        nsl = slice(n0, n0 + nsz)
                        nc.vector.tensor_add(ss[:, nsl], xs[:, 0, nsl], xs[:, 1, nsl])
                else:
                    nc.vector.tensor_add(ss[:], xs[:, 0], xs[:, 1])

                last = si == n_seq - 1
                for m in range(C_OUT // 128):
                    msl = slice(m * 128, (m + 1) * 128)
                    ps = pp.tile([128, 2, 512], f32, tag="ps")
                    for ci, (n0, nsz) in enumerate(N_CHUNKS):
                        nc.tensor.matmul(
                            ps[:, ci, 0:nsz],
                            tt2[:, msl],
                            ss[:, n0 : n0 + nsz],
                            start=True,
                            stop=True,
                        )
                    ysm = io.tile([128, S], fp16, tag="ysm", bufs=6)
                    ysv = ysm.rearrange("p (c n) -> p c n", c=2)
                    if last:
                        # drain fast: per-chunk copies and stores spread over
                        # engines/queues so the tail is one half-tile long
                        cengs = ("v", "s") if m % 2 == 0 else ("s", "v")
                        qouts = ("y", "s", "g", "y", "s", "g", "y", "s")
                        for ci in range(2):
                            dst = ysv[:, ci, 0:392]
                            src = ps[:, ci, 0:392]
                            if cengs[ci] == "v":
                                nc.vector.tensor_copy(dst, src)
                            else:
                                nc.scalar.copy(dst, src)
                            q(qouts[m * 2 + ci]).dma_start(
                                out=y[s, msl, ci * 392 : (ci + 1) * 392],
                                in_=ysm[:, ci * 392 : (ci + 1) * 392],
                            )
                    else:
                        dst = ysv[:, :, 0:392]
                        src = ps[:, :, 0:392]
                        eng = _COPY_ENG[s * 4 + m]
                        if eng == "v":
                            nc.vector.tensor_copy(dst, src)
                        else:
                            nc.scalar.copy(dst, src)
                        q(_OUT_Q[s * 4 + m]).dma_start(out=y[s, msl, :], in_=ysm[:])

    nc.compile()
    return nc


# ------------------------------------------------------- fp16_lo (old) variant


def _build_fp16lo(reps=1):
    import concourse.mybir as mybir
    from concourse import bacc
    from concourse.tile import TileContext

    f32 = mybir.dt.float32
    bf16 = mybir.dt.bfloat16
    fp16 = mybir.dt.float16

    nc = bacc.Bacc(None, target_bir_lowering=False)
    x = nc.dram_tensor("x", (BS, C_IN, S), f32, kind="ExternalInput")
    tt = nc.dram_tensor("tt", (C_IN, C_OUT), bf16, kind="ExternalInput")
    y = nc.dram_tensor("y", (BS, C_OUT, S), f32, kind="ExternalOutput")

    with TileContext(nc) as tc:
        with (
            tc.tile_pool(name="w", bufs=1) as wp,
            tc.tile_pool(name="io", bufs=3) as io,
            tc.tile_pool(name="ps", bufs=8, space="PSUM") as pp,
        ):
            tts = []
            tth = []
            for k in range(2):
                t = wp.tile([128, C_OUT], bf16, tag=f"tt{k}")
                nc.sync.dma_start(out=t[:], in_=tt[k * 128 : (k + 1) * 128, :])
                tts.append(t)
                h = wp.tile([128, C_OUT], fp16, tag=f"tth{k}")
                nc.vector.tensor_copy(h[:], t[:])
                tth.append(h)

            sample_seq = [s for _ in range(reps) for s in range(BS)]
            for si, s in enumerate(sample_seq):
                last_sample = si == len(sample_seq) - 1
                first_sample = si == 0
                xsk = []
                for k in range(2):
                    xs = io.tile([128, S], f32, tag="xs", bufs=8)
                    if first_sample:
                        for n0, nsz in N_CHUNKS:
                            nc.scalar.dma_start(
                                out=xs[:, n0 : n0 + nsz],
                                in_=x[s, k * 128 : (k + 1) * 128, n0 : n0 + nsz],
                            )
                    else:
                        nc.scalar.dma_start(
                            out=xs[:], in_=x[s, k * 128 : (k + 1) * 128, :]
                        )
                    xsk.append(xs)

                xhk = [
                    xs.bitcast(bf16).rearrange("p (f two) -> p f two", two=2)[:, :, 1]
                    for xs in xsk
                ]
                xlk = []
                for k in range(2):
                    xl = io.tile([128, S], fp16, tag="xl", bufs=8)
                    if first_sample:
                        for n0, nsz in N_CHUNKS:
                            nsl = slice(n0, n0 + nsz)
                            nc.vector.tensor_sub(
                                xl[:, nsl], xsk[k][:, nsl], xhk[k][:, nsl]
                            )
                    else:
                        nc.vector.tensor_sub(xl[:], xsk[k][:], xhk[k])
                    xlk.append(xl)
                passes = [
                    (xlk[0][:], tth[0]),
                    (xhk[0], tts[0]),
                    (xlk[1][:], tth[1]),
                    (xhk[1], tts[1]),
                ]

                for m in range(C_OUT // 128):
                    msl = slice(m * 128, (m + 1) * 128)
                    ysm = io.tile([128, S], f32, tag="ysm", bufs=6)
                    for ni, (n0, nsz) in enumerate(N_CHUNKS):
                        nsl = slice(n0, n0 + nsz)
                        ps = pp.tile([128, nsz], f32, tag="ps")
                        for i, (src, w) in enumerate(passes):
                            nc.tensor.matmul(
                                ps[:],
                                w[:, msl],
                                src[:, nsl],
                                start=(i == 0),
                                stop=(i == len(passes) - 1),
                            )
                        dst = ysm[:, nsl]
                        if last_sample:
                            if ni == 0:
                                nc.vector.tensor_copy(dst, ps[:])
                                nc.sync.dma_start(out=y[s, msl, nsl], in_=dst)
                            else:
                                nc.scalar.copy(dst, ps[:])
                                nc.scalar.dma_start(out=y[s, msl, nsl], in_=dst)
                        elif (m * len(N_CHUNKS) + ni) % 2 == 0:
                            nc.vector.tensor_copy(dst, ps[:])
                        else:
                            nc.scalar.copy(dst, ps[:])
                    if not last_sample:
                        nc.sync.dma_start(out=y[s, msl, :], in_=ysm[:])

    nc.compile()
    return nc


_cache = {}


def _get_nc(variant=None, reps=1):
    variant = variant or VARIANT
    key = (variant, reps)
    if key not in _cache:
        if variant == "t2":
            _cache[key] = _build_t2(reps)
        elif variant == "fp16_lo":
            _cache[key] = _build_fp16lo(reps)
        else:
            raise ValueError(variant)
    return _cache[key]


def _in_maps(x_np, variant):
    T = _dwht_T()
    if variant == "t2":
        tt_np = np.ascontiguousarray(T[:, ::2].T).astype(np.float16)  # (128, 512)
        x16 = x_np.astype(np.float16)
        return [
            {"x": x16[i * BS : (i + 1) * BS], "tt": tt_np} for i in range(N_CORES)
        ]
    import ml_dtypes

    ttT = np.ascontiguousarray(T.T)  # (256, 512), lhsT layout
    tt_np = ttT.astype(ml_dtypes.bfloat16)
    return [
        {"x": x_np[i * BS : (i + 1) * BS], "tt": tt_np} for i in range(N_CORES)
    ]


def _run(x_np, variant=None, trace=False, reps=1):
    from concourse.bass_utils import run_bass_kernel_spmd

    variant = variant or VARIANT
    nc = _get_nc(variant, reps)
    res = run_bass_kernel_spmd(
        nc, _in_maps(x_np, variant), list(range(N_CORES)), trace=trace
    )
    y = np.stack([r["y"] for r in res.results]).astype(np.float32)
    y = y.reshape(B, C_OUT, HH, WW)
    return y, res


def kernel(x: np.ndarray) -> np.ndarray:
    x_np = np.ascontiguousarray(np.asarray(x), dtype=np.float32).reshape(B, C_IN, S)
    y, _ = _run(x_np)
    return y


# revision 18
# speedup vs baseline: 1.8218x; 1.0430x over previous
"""DWHT (buggy in-place Walsh-Hadamard channel transform + channel shuffle) on 8 trn2 cores.

The whole nn.Module is a fixed linear map on the channel axis:
    y[b, :, h, w] = T @ x[b, :, h, w]
with T a (512, 256) matrix of small integers (|T| <= 13, exact in fp16).

Structure exploited (variant "t2", default): every row of T is constant on
input-channel pairs, i.e. T = T2 @ A with A the pair-sum map
(s[c] = x[2c] + x[2c+1]) and T2 = T[:, ::2] a (512, 128) small-integer
matrix.  Per sample the kernel therefore:
  1. loads x as fp16 [128 pairs, 2, 784] (one DMA),
  2. computes s = x[:,0] + x[:,1] on DVE (fp16, 2x mode),
  3. runs 8 single-k matmuls T2 @ s (k=128, fp16, exact) into PSUM,
  4. copies PSUM fp32 -> SBUF fp16 (balanced across DVE and ACT; gpsimd
     cannot read PSUM, the BIR verifier rejects it),
  5. DMAs fp16 y tiles out (balanced across the SP/ACT/Pool queues).
Batch 64 is sharded 8-ways (data parallel, 8 samples/core).

Precision: x is rounded to fp16 on the host (T2 and the pair-sum stay exact
in fp16; PSUM accumulates fp32), output is written fp16 and widened to fp32
on the host: measured rel err ~1e-4 vs the fp32 reference, far inside the
2e-2 gate.  The kernel is memory/copy bound: ~9.6 MB of fp16 HBM traffic
per core spread over the three DMA-capable queues, with the PSUM->SBUF
downcast copies load-balanced against them.
"""

import os
import sys

import numpy as np

for _p in ("/opt/trn_rl_repo", "/root/.axon_site/_ro/trn_rl_repo"):
    if os.path.isdir(_p) and _p not in sys.path:
        sys.path.append(_p)

B, C_IN, C_OUT, HH, WW = 64, 256, 512, 28, 28
S = HH * WW  # 784
N_CORES = 8
BS = B // N_CORES  # 8 samples per core
N_PASSES, GROUPS = 8, 8

VARIANT = os.environ.get("DWHT_VARIANT", "t2")

# spatial split per PSUM bank (each chunk <= 512 fp32 = one bank)
N_CHUNKS = ((0, 392), (392, 392))


def _dwht_T() -> np.ndarray:
    """Build the (512, 256) transform matrix by running the reference
    butterfly (including its partial-update in-place semantics) on identity."""
    x = np.zeros((C_OUT, C_IN), np.float64)
    x[:C_IN] = np.eye(C_IN)
    half = C_OUT // 2
    for _ in range(N_PASSES):
        top = x[::2] + x[1::2]
        x = x.copy()
        x[:half] = top
        bottom = x[::2] - x[1::2]
        x[half:] = bottom
    # channel shuffle with groups=8
    x = x.reshape(GROUPS, C_OUT // GROUPS, C_IN).transpose(1, 0, 2).reshape(C_OUT, C_IN)
    return x


# ---------------------------------------------------------------- t2 variant

# engine assignment tables, tuned against the CoreSim cost model.
# copy engines per (sample, m-tile): v=vector(DVE) s=scalar(ACT) g=gpsimd(Pool)
# out-dma queues per (sample, m-tile): y=sync(SP) s=scalar(ACT) g=gpsimd(Pool)
# in-dma queue per sample.
# Balance rationale: PSUM->SBUF copies may only run on DVE and ACT (gpsimd
# has no PSUM access), so the SP and Pool queues carry nearly all DMA
# traffic and ACT's queue share goes to copies instead.
_COPY_ENG = os.environ.get(
    "DWHT_COPY", "vsvs" + "vsvs" + "vsvs" + "vsvs" + "vsvs" + "vsss" + "vsss" + "vsvs"
)
_OUT_Q = os.environ.get(
    "DWHT_OUTQ", "ygyg" + "ygyy" + "ygyg" + "ygyy" + "ygyg" + "ygyg" + "ygyg" + "ygys"
)
_IN_Q = os.environ.get("DWHT_INQ", "-ygygygg")


def _build_t2(reps=1):
    import concourse.mybir as mybir
    from concourse import bacc
    from concourse.tile import TileContext

    f32 = mybir.dt.float32
    fp16 = mybir.dt.float16

    nc = bacc.Bacc(None, target_bir_lowering=False)
    x = nc.dram_tensor("x", (BS, C_IN, S), fp16, kind="ExternalInput")
    tt = nc.dram_tensor("tt", (C_IN // 2, C_OUT), fp16, kind="ExternalInput")
    y = nc.dram_tensor("y", (BS, C_OUT, S), fp16, kind="ExternalOutput")

    def q(ch):
        return {"y": nc.sync, "s": nc.scalar, "g": nc.gpsimd, "v": nc.vector}[ch]

    with TileContext(nc) as tc:
        with (
            tc.tile_pool(name="w", bufs=1) as wp,
            tc.tile_pool(name="io", bufs=3) as io,
            tc.tile_pool(name="ps", bufs=4, space="PSUM") as pp,
        ):
            tt2 = wp.tile([128, C_OUT], fp16, tag="tt2")
            nc.sync.dma_start(out=tt2[:], in_=tt[:, :])

            sample_seq = [s for _ in range(reps) for s in range(BS)]
            n_seq = len(sample_seq)
            xss = {}
            # all input loads upfront: input traffic is the only DMA work
            # that exists before the first PSUM tile is ready, so it must
            # fill the queues' lead-in window
            for si in range(n_seq):
                s = sample_seq[si]
                xs = io.tile([128, 2, S], fp16, tag="xs", bufs=BS)
                src = x[s].rearrange("(p two) f -> p two f", two=2)
                if si == 0:
                    # split the first load across two queues so the fill
                    # critical path is one half-transfer, not a full one
                    nc.sync.dma_start(out=xs[:, :, 0:392], in_=src[:, :, 0:392])
                    nc.gpsimd.dma_start(out=xs[:, :, 392:S], in_=src[:, :, 392:S])
                else:
                    q(_IN_Q[s]).dma_start(out=xs[:], in_=src)
                xss[si] = xs

            for si, s in enumerate(sample_seq):
                xs = xss.pop(si)
                # pair-sum on DVE (fp16, packed, SBUF -> 2x mode); split for
                # the first sample so matmuls can start on the first half
                ss = io.tile([128, S], fp16, tag="ss", bufs=3)
                if si == 0:
                    for n0, nsz in N_CHUNKS:
                        nsl = slice(n0, n0 + nsz)
                        nc.vector.tensor_add(ss[:, nsl], xs[:, 0, nsl], xs[:, 1, nsl])
                elif s % 2 == 1:
                    nc.gpsimd.tensor_add(ss[:], xs[:, 0], xs[:, 1])
                else:
                    nc.vector.tensor_add(ss[:], xs[:, 0], xs[:, 1])

                last = si == n_seq - 1
                for m in range(C_OUT // 128):
                    msl = slice(m * 128, (m + 1) * 128)
                    ps = pp.tile([128, 2, 512], f32, tag="ps")
                    for ci, (n0, nsz) in enumerate(N_CHUNKS):
                        nc.tensor.matmul(
                            ps[:, ci, 0:nsz],
                            tt2[:, msl],
                            ss[:, n0 : n0 + nsz],
                            start=True,
                            stop=True,
                        )
                    ysm = io.tile([128, S], fp16, tag="ysm", bufs=6)
                    ysv = ysm.rearrange("p (c n) -> p c n", c=2)
                    if last:
                        # drain fast: per-chunk copies and stores spread over
                        # engines/queues so the tail is one half-tile long
                        cengs = ("v", "s") if m % 2 == 0 else ("s", "v")
                        qouts = ("y", "s", "g", "y", "s", "g", "y", "s")
                        for ci in range(2):
                            dst = ysv[:, ci, 0:392]
                            src = ps[:, ci, 0:392]
                            if cengs[ci] == "v":
                                nc.vector.tensor_copy(dst, src)
                            else:
                                nc.scalar.copy(dst, src)
                            q(qouts[m * 2 + ci]).dma_start(
                                out=y[s, msl, ci * 392 : (ci + 1) * 392],
                                in_=ysm[:, ci * 392 : (ci + 1) * 392],
                            )
                    else:
                        dst = ysv[:, :, 0:392]
                        src = ps[:, :, 0:392]
                        eng = _COPY_ENG[s * 4 + m]
                        if eng == "v":
                            nc.vector.tensor_copy(dst, src)
                        else:
                            nc.scalar.copy(dst, src)
                        q(_OUT_Q[s * 4 + m]).dma_start(out=y[s, msl, :], in_=ysm[:])

    nc.compile()
    return nc


# ------------------------------------------------------- fp16_lo (old) variant


def _build_fp16lo(reps=1):
    import concourse.mybir as mybir
    from concourse import bacc
    from concourse.tile import TileContext

    f32 = mybir.dt.float32
    bf16 = mybir.dt.bfloat16
    fp16 = mybir.dt.float16

    nc = bacc.Bacc(None, target_bir_lowering=False)
    x = nc.dram_tensor("x", (BS, C_IN, S), f32, kind="ExternalInput")
    tt = nc.dram_tensor("tt", (C_IN, C_OUT), bf16, kind="ExternalInput")
    y = nc.dram_tensor("y", (BS, C_OUT, S), f32, kind="ExternalOutput")

    with TileContext(nc) as tc:
        with (
            tc.tile_pool(name="w", bufs=1) as wp,
            tc.tile_pool(name="io", bufs=3) as io,
            tc.tile_pool(name="ps", bufs=8, space="PSUM") as pp,
        ):
            tts = []
            tth = []
            for k in range(2):
                t = wp.tile([128, C_OUT], bf16, tag=f"tt{k}")
                nc.sync.dma_start(out=t[:], in_=tt[k * 128 : (k + 1) * 128, :])
                tts.append(t)
                h = wp.tile([128, C_OUT], fp16, tag=f"tth{k}")
                nc.vector.tensor_copy(h[:], t[:])
                tth.append(h)

            sample_seq = [s for _ in range(reps) for s in range(BS)]
            for si, s in enumerate(sample_seq):
                last_sample = si == len(sample_seq) - 1
                first_sample = si == 0
                xsk = []
                for k in range(2):
                    xs = io.tile([128, S], f32, tag="xs", bufs=8)
                    if first_sample:
                        for n0, nsz in N_CHUNKS:
                            nc.scalar.dma_start(
                                out=xs[:, n0 : n0 + nsz],
                                in_=x[s, k * 128 : (k + 1) * 128, n0 : n0 + nsz],
                            )
                    else:
                        nc.scalar.dma_start(
                            out=xs[:], in_=x[s, k * 128 : (k + 1) * 128, :]
                        )
                    xsk.append(xs)

                xhk = [
                    xs.bitcast(bf16).rearrange("p (f two) -> p f two", two=2)[:, :, 1]
                    for xs in xsk
                ]
                xlk = []
                for k in range(2):
                    xl = io.tile([128, S], fp16, tag="xl", bufs=8)
                    if first_sample:
                        for n0, nsz in N_CHUNKS:
                            nsl = slice(n0, n0 + nsz)
                            nc.vector.tensor_sub(
                                xl[:, nsl], xsk[k][:, nsl], xhk[k][:, nsl]
                            )
                    else:
                        nc.vector.tensor_sub(xl[:], xsk[k][:], xhk[k])
                    xlk.append(xl)
                passes = [
                    (xlk[0][:], tth[0]),
                    (xhk[0], tts[0]),
                    (xlk[1][:], tth[1]),
                    (xhk[1], tts[1]),
                ]

                for m in range(C_OUT // 128):
                    msl = slice(m * 128, (m + 1) * 128)
                    ysm = io.tile([128, S], f32, tag="ysm", bufs=6)
                    for ni, (n0, nsz) in enumerate(N_CHUNKS):
                        nsl = slice(n0, n0 + nsz)
                        ps = pp.tile([128, nsz], f32, tag="ps")
                        for i, (src, w) in enumerate(passes):
                            nc.tensor.matmul(
                                ps[:],
                                w[:, msl],
                                src[:, nsl],
                                start=(i == 0),
                                stop=(i == len(passes) - 1),
                            )
                        dst = ysm[:, nsl]
                        if last_sample:
                            if ni == 0:
                                nc.vector.tensor_copy(dst, ps[:])
                                nc.sync.dma_start(out=y[s, msl, nsl], in_=dst)
                            else:
                                nc.scalar.copy(dst, ps[:])
                                nc.scalar.dma_start(out=y[s, msl, nsl], in_=dst)
                        elif (m * len(N_CHUNKS) + ni) % 2 == 0:
                            nc.vector.tensor_copy(dst, ps[:])
                        else:
                            nc.scalar.copy(dst, ps[:])
                    if not last_sample:
                        nc.sync.dma_start(out=y[s, msl, :], in_=ysm[:])

    nc.compile()
    return nc


_cache = {}


def _get_nc(variant=None, reps=1):
    variant = variant or VARIANT
    key = (variant, reps)
    if key not in _cache:
        if variant == "t2":
            _cache[key] = _build_t2(reps)
        elif variant == "fp16_lo":
            _cache[key] = _build_fp16lo(reps)
        else:
            raise ValueError(variant)
    return _cache[key]


def _in_maps(x_np, variant):
    T = _dwht_T()
    if variant == "t2":
        tt_np = np.ascontiguousarray(T[:, ::2].T).astype(np.float16)  # (128, 512)
        x16 = x_np.astype(np.float16)
        return [
            {"x": x16[i * BS : (i + 1) * BS], "tt": tt_np} for i in range(N_CORES)
        ]
    import ml_dtypes

    ttT = np.ascontiguousarray(T.T)  # (256, 512), lhsT layout
    tt_np = ttT.astype(ml_dtypes.bfloat16)
    return [
        {"x": x_np[i * BS : (i + 1) * BS], "tt": tt_np} for i in range(N_CORES)
    ]


def _run(x_np, variant=None, trace=False, reps=1):
    from concourse.bass_utils import run_bass_kernel_spmd

    variant = variant or VARIANT
    nc = _get_nc(variant, reps)
    res = run_bass_kernel_spmd(
        nc, _in_maps(x_np, variant), list(range(N_CORES)), trace=trace
    )
    y = np.stack([r["y"] for r in res.results]).astype(np.float32)
    y = y.reshape(B, C_OUT, HH, WW)
    return y, res


def kernel(x: np.ndarray) -> np.ndarray:
    x_np = np.ascontiguousarray(np.asarray(x), dtype=np.float32).reshape(B, C_IN, S)
    y, _ = _run(x_np)
    return y


# revision 20
# speedup vs baseline: 1.8959x; 1.0407x over previous
"""DWHT (buggy in-place Walsh-Hadamard channel transform + channel shuffle) on 8 trn2 cores.

The whole nn.Module is a fixed linear map on the channel axis:
    y[b, :, h, w] = T @ x[b, :, h, w]
with T a (512, 256) matrix of small integers (|T| <= 13, exact in fp16).

Structure exploited (variant "t2", default): every row of T is constant on
input-channel pairs, i.e. T = T2 @ A with A the pair-sum map
(s[c] = x[2c] + x[2c+1]) and T2 = T[:, ::2] a (512, 128) small-integer
matrix.  Per sample the kernel therefore:
  1. loads x as fp16 [128 pairs, 2, 784] (one DMA),
  2. computes s = x[:,0] + x[:,1] (fp16; DVE 2x mode for even samples,
     gpsimd for odd ones to spread the elementwise load),
  3. runs 8 single-k matmuls T2 @ s (k=128, fp16, exact) into PSUM,
  4. copies PSUM fp32 -> SBUF fp16 (balanced across DVE and ACT; gpsimd
     cannot read PSUM, the BIR verifier rejects it),
  5. DMAs fp16 y tiles out (balanced across the SP/ACT/Pool queues).
Batch 64 is sharded 8-ways (data parallel, 8 samples/core).

Precision: x is rounded to fp16 on the host (T2 and the pair-sum stay exact
in fp16; PSUM accumulates fp32), output is written fp16 and widened to fp32
on the host: measured rel err ~1e-4 vs the fp32 reference, far inside the
2e-2 gate.  The kernel is memory/copy bound: ~9.6 MB of fp16 HBM traffic
per core spread over the three DMA-capable queues, with the PSUM->SBUF
downcast copies load-balanced against them.
"""

import os
import sys

import numpy as np

for _p in ("/opt/trn_rl_repo", "/root/.axon_site/_ro/trn_rl_repo"):
    if os.path.isdir(_p) and _p not in sys.path:
        sys.path.append(_p)

B, C_IN, C_OUT, HH, WW = 64, 256, 512, 28, 28
S = HH * WW  # 784
N_CORES = 8
BS = B // N_CORES  # 8 samples per core
N_PASSES, GROUPS = 8, 8

VARIANT = os.environ.get("DWHT_VARIANT", "t2")

# spatial split per PSUM bank (each chunk <= 512 fp32 = one bank)
N_CHUNKS = ((0, 392), (392, 392))


def _dwht_T() -> np.ndarray:
    """Build the (512, 256) transform matrix by running the reference
    butterfly (including its partial-update in-place semantics) on identity."""
    x = np.zeros((C_OUT, C_IN), np.float64)
    x[:C_IN] = np.eye(C_IN)
    half = C_OUT // 2
    for _ in range(N_PASSES):
        top = x[::2] + x[1::2]
        x = x.copy()
        x[:half] = top
        bottom = x[::2] - x[1::2]
        x[half:] = bottom
    # channel shuffle with groups=8
    x = x.reshape(GROUPS, C_OUT // GROUPS, C_IN).transpose(1, 0, 2).reshape(C_OUT, C_IN)
    return x


# ---------------------------------------------------------------- t2 variant

# engine assignment tables, tuned against the CoreSim cost model.
# copy engines per (sample, m-tile): v=vector(DVE) s=scalar(ACT) g=gpsimd(Pool)
# out-dma queues per (sample, m-tile): y=sync(SP) s=scalar(ACT) g=gpsimd(Pool)
# in-dma queue per sample.
# Balance rationale: PSUM->SBUF copies may only run on DVE and ACT (gpsimd
# has no PSUM access), so the SP and Pool queues carry nearly all DMA
# traffic and ACT's queue share goes to copies instead.
_COPY_ENG = os.environ.get(
    "DWHT_COPY", "vsvs" + "vsvs" + "vsvs" + "vsvs" + "vsvs" + "vsss" + "vsss" + "vsvs"
)
_OUT_Q = os.environ.get(
    "DWHT_OUTQ", "ygyg" + "ygyy" + "ygyg" + "ygyy" + "ygyg" + "ygyg" + "ygyg" + "ygys"
)
_IN_Q = os.environ.get("DWHT_INQ", "-ygygygg")


def _build_t2(reps=1):
    import concourse.mybir as mybir
    from concourse import bacc
    from concourse.tile import TileContext

    f32 = mybir.dt.float32
    fp16 = mybir.dt.float16

    nc = bacc.Bacc(None, target_bir_lowering=False)
    x = nc.dram_tensor("x", (BS, C_IN, S), fp16, kind="ExternalInput")
    tt = nc.dram_tensor("tt", (C_IN // 2, C_OUT), fp16, kind="ExternalInput")
    y = nc.dram_tensor("y", (BS, C_OUT, S), fp16, kind="ExternalOutput")

    def q(ch):
        return {"y": nc.sync, "s": nc.scalar, "g": nc.gpsimd, "v": nc.vector}[ch]

    with TileContext(nc) as tc:
        with (
            tc.tile_pool(name="w", bufs=1) as wp,
            tc.tile_pool(name="io", bufs=3) as io,
            tc.tile_pool(name="ps", bufs=4, space="PSUM") as pp,
        ):
            tt2 = wp.tile([128, C_OUT], fp16, tag="tt2")
            nc.sync.dma_start(out=tt2[:], in_=tt[:, :])

            sample_seq = [s for _ in range(reps) for s in range(BS)]
            n_seq = len(sample_seq)
            xss = {}
            # all input loads upfront: input traffic is the only DMA work
            # that exists before the first PSUM tile is ready, so it must
            # fill the queues' lead-in window
            for si in range(n_seq):
                s = sample_seq[si]
                xs = io.tile([128, 2, S], fp16, tag="xs", bufs=BS)
                src = x[s].rearrange("(p two) f -> p two f", two=2)
                if si == 0:
                    # split the first load across two queues so the fill
                    # critical path is one half-transfer, not a full one
                    nc.sync.dma_start(out=xs[:, :, 0:392], in_=src[:, :, 0:392])
                    nc.gpsimd.dma_start(out=xs[:, :, 392:S], in_=src[:, :, 392:S])
                else:
                    q(_IN_Q[s]).dma_start(out=xs[:], in_=src)
                xss[si] = xs

            for si, s in enumerate(sample_seq):
                xs = xss.pop(si)
                # pair-sum on DVE (fp16, packed, SBUF -> 2x mode); split for
                # the first sample so matmuls can start on the first half
                ss = io.tile([128, S], fp16, tag="ss", bufs=4)
                if si == 0:
                    for n0, nsz in N_CHUNKS:
                        nsl = slice(n0, n0 + nsz)
                        nc.vector.tensor_add(ss[:, nsl], xs[:, 0, nsl], xs[:, 1, nsl])
                elif s % 2 == 1:
                    with tc.high_priority():
                        nc.gpsimd.tensor_add(ss[:], xs[:, 0], xs[:, 1])
                else:
                    with tc.high_priority():
                        nc.vector.tensor_add(ss[:], xs[:, 0], xs[:, 1])

                last = si == n_seq - 1
                for m in range(C_OUT // 128):
                    msl = slice(m * 128, (m + 1) * 128)
                    ps = pp.tile([128, 2, 512], f32, tag="ps")
                    for ci, (n0, nsz) in enumerate(N_CHUNKS):
                        nc.tensor.matmul(
                            ps[:, ci, 0:nsz],
                            tt2[:, msl],
                            ss[:, n0 : n0 + nsz],
                            start=True,
                            stop=True,
                        )
                    ysm = io.tile([128, S], fp16, tag="ysm", bufs=8)
                    ysv = ysm.rearrange("p (c n) -> p c n", c=2)
                    if last:
                        # drain fast: per-chunk copies and stores spread over
                        # engines/queues so the tail is one half-tile long
                        cengs = ("v", "s") if m % 2 == 0 else ("s", "v")
                        qouts = ("y", "s", "g", "y", "s", "g", "y", "s")
                        for ci in range(2):
                            dst = ysv[:, ci, 0:392]
                            src = ps[:, ci, 0:392]
                            if cengs[ci] == "v":
                                nc.vector.tensor_copy(dst, src)
                            else:
                                nc.scalar.copy(dst, src)
                            q(qouts[m * 2 + ci]).dma_start(
                                out=y[s, msl, ci * 392 : (ci + 1) * 392],
                                in_=ysm[:, ci * 392 : (ci + 1) * 392],
                            )
                    else:
                        dst = ysv[:, :, 0:392]
                        src = ps[:, :, 0:392]
                        eng = _COPY_ENG[s * 4 + m]
                        if eng == "v":
                            nc.vector.tensor_copy(dst, src)
                        else:
                            nc.scalar.copy(dst, src)
                        q(_OUT_Q[s * 4 + m]).dma_start(out=y[s, msl, :], in_=ysm[:])

    nc.compile()
    return nc


# ------------------------------------------------------- fp16_lo (old) variant


def _build_fp16lo(reps=1):
    import concourse.mybir as mybir
    from concourse import bacc
    from concourse.tile import TileContext

    f32 = mybir.dt.float32
    bf16 = mybir.dt.bfloat16
    fp16 = mybir.dt.float16

    nc = bacc.Bacc(None, target_bir_lowering=False)
    x = nc.dram_tensor("x", (BS, C_IN, S), f32, kind="ExternalInput")
    tt = nc.dram_tensor("tt", (C_IN, C_OUT), bf16, kind="ExternalInput")
    y = nc.dram_tensor("y", (BS, C_OUT, S), f32, kind="ExternalOutput")

    with TileContext(nc) as tc:
        with (
            tc.tile_pool(name="w", bufs=1) as wp,
            tc.tile_pool(name="io", bufs=3) as io,
            tc.tile_pool(name="ps", bufs=8, space="PSUM") as pp,
        ):
            tts = []
            tth = []
            for k in range(2):
                t = wp.tile([128, C_OUT], bf16, tag=f"tt{k}")
                nc.sync.dma_start(out=t[:], in_=tt[k * 128 : (k + 1) * 128, :])
                tts.append(t)
                h = wp.tile([128, C_OUT], fp16, tag=f"tth{k}")
                nc.vector.tensor_copy(h[:], t[:])
                tth.append(h)

            sample_seq = [s for _ in range(reps) for s in range(BS)]
            for si, s in enumerate(sample_seq):
                last_sample = si == len(sample_seq) - 1
                first_sample = si == 0
                xsk = []
                for k in range(2):
                    xs = io.tile([128, S], f32, tag="xs", bufs=8)
                    if first_sample:
                        for n0, nsz in N_CHUNKS:
                            nc.scalar.dma_start(
                                out=xs[:, n0 : n0 + nsz],
                                in_=x[s, k * 128 : (k + 1) * 128, n0 : n0 + nsz],
                            )
                    else:
                        nc.scalar.dma_start(
                            out=xs[:], in_=x[s, k * 128 : (k + 1) * 128, :]
                        )
                    xsk.append(xs)

                xhk = [
                    xs.bitcast(bf16).rearrange("p (f two) -> p f two", two=2)[:, :, 1]
                    for xs in xsk
                ]
                xlk = []
                for k in range(2):
                    xl = io.tile([128, S], fp16, tag="xl", bufs=8)
                    if first_sample:
                        for n0, nsz in N_CHUNKS:
                            nsl = slice(n0, n0 + nsz)
                            nc.vector.tensor_sub(
                                xl[:, nsl], xsk[k][:, nsl], xhk[k][:, nsl]
                            )
                    else:
                        nc.vector.tensor_sub(xl[:], xsk[k][:], xhk[k])
                    xlk.append(xl)
                passes = [
                    (xlk[0][:], tth[0]),
                    (xhk[0], tts[0]),
                    (xlk[1][:], tth[1]),
                    (xhk[1], tts[1]),
                ]

                for m in range(C_OUT // 128):
                    msl = slice(m * 128, (m + 1) * 128)
                    ysm = io.tile([128, S], f32, tag="ysm", bufs=6)
                    for ni, (n0, nsz) in enumerate(N_CHUNKS):
                        nsl = slice(n0, n0 + nsz)
                        ps = pp.tile([128, nsz], f32, tag="ps")
                        for i, (src, w) in enumerate(passes):
                            nc.tensor.matmul(
                                ps[:],
                                w[:, msl],
                                src[:, nsl],
                                start=(i == 0),
                                stop=(i == len(passes) - 1),
                            )
                        dst = ysm[:, nsl]
                        if last_sample:
                            if ni == 0:
                                nc.vector.tensor_copy(dst, ps[:])
                                nc.sync.dma_start(out=y[s, msl, nsl], in_=dst)
                            else:
                                nc.scalar.copy(dst, ps[:])
                                nc.scalar.dma_start(out=y[s, msl, nsl], in_=dst)
                        elif (m * len(N_CHUNKS) + ni) % 2 == 0:
                            nc.vector.tensor_copy(dst, ps[:])
                        else:
                            nc.scalar.copy(dst, ps[:])
                    if not last_sample:
                        nc.sync.dma_start(out=y[s, msl, :], in_=ysm[:])

    nc.compile()
    return nc


_cache = {}


def _get_nc(variant=None, reps=1):
    variant = variant or VARIANT
    key = (variant, reps)
    if key not in _cache:
        if variant == "t2":
            _cache[key] = _build_t2(reps)
        elif variant == "fp16_lo":
            _cache[key] = _build_fp16lo(reps)
        else:
            raise ValueError(variant)
    return _cache[key]


def _in_maps(x_np, variant):
    T = _dwht_T()
    if variant == "t2":
        tt_np = np.ascontiguousarray(T[:, ::2].T).astype(np.float16)  # (128, 512)
        x16 = x_np.astype(np.float16)
        return [
            {"x": x16[i * BS : (i + 1) * BS], "tt": tt_np} for i in range(N_CORES)
        ]
    import ml_dtypes

    ttT = np.ascontiguousarray(T.T)  # (256, 512), lhsT layout
    tt_np = ttT.astype(ml_dtypes.bfloat16)
    return [
        {"x": x_np[i * BS : (i + 1) * BS], "tt": tt_np} for i in range(N_CORES)
    ]


def _run(x_np, variant=None, trace=False, reps=1):
    from concourse.bass_utils import run_bass_kernel_spmd

    variant = variant or VARIANT
    nc = _get_nc(variant, reps)
    res = run_bass_kernel_spmd(
        nc, _in_maps(x_np, variant), list(range(N_CORES)), trace=trace
    )
    y = np.stack([r["y"] for r in res.results]).astype(np.float32)
    y = y.reshape(B, C_OUT, HH, WW)
    return y, res


def kernel(x: np.ndarray) -> np.ndarray:
    x_np = np.ascontiguousarray(np.asarray(x), dtype=np.float32).reshape(B, C_IN, S)
    y, _ = _run(x_np)
    return y
